# revision 26
# baseline (speedup 1.0000x reference)
"""Bipartite graph multi-head attention (GAT-style) on 8 TRN2 NeuronCores.

Math (per batch b, head h, direction fe shown; ef symmetric):
    F1 = F0 @ Wf[h]; E1 = E0 @ We[h]
    s[n] = F1[n]·a;  t[m] = E1[m]·a
    u = s[n] + t[m];  e = leaky_relu(u, 0.2) masked by adj
    alpha = softmax_m(e);  out = alpha @ E1

Key identity: exp(lrelu(u)) = exp(0.2u)·max(exp(0.8u), 1); the row
factor exp(0.2 s[n]) cancels in the softmax ratio, so the unnormalized
weight (transposed, contraction dim on partitions) is
    G[m,n] = adjT[m,n] · max(es8[n]·exp(t[m]), exp(0.2·t[m]))
with es8 = exp(0.8 s).  One tensor_scalar (two fused ALU ops with
per-partition scalars) + one tensor_tensor against the adjacency per
score element; numerator and denominator come from a PE matmul against
[E1 | 1]; out = numer / denom.

Sharding: fe output rows n (2048) and ef output rows m (4096) split 8
ways; adjacency slices are the only large per-core inputs; projections
are computed replicated (no collectives).
"""

import os
import sys

import numpy as np

if "/opt/trn_rl_repo" not in sys.path:
    sys.path.insert(0, "/opt/trn_rl_repo")

import ml_dtypes

import concourse.bass as bass
import concourse.tile as tile
from concourse import bacc, mybir
from concourse.bass_utils import run_bass_kernel_spmd

BF16 = mybir.dt.bfloat16
F32 = mybir.dt.float32
AF = mybir.ActivationFunctionType
ALU = mybir.AluOpType

B, N_F, N_E = 2, 2048, 4096
FDIM, HDIM, HEADS = 128, 64, 4
NCORES = 8
NFC = N_F // NCORES  # 256 fe rows per core
NEC = N_E // NCORES  # 512 ef rows per core
MCH = N_E // 128  # 32 m-chunks (fe contraction)
NCH = N_F // 128  # 16 n-chunks (ef contraction)
W = HDIM + 1  # 65: [dst feature | ones] columns
BW = HEADS * W  # 260: bundled projection width
NI_FE = NFC // 128  # 2 output row tiles (fe)
NI_EF = NEC // 128  # 4 output row tiles (ef)


def build_program():
    nc = bacc.Bacc("TRN2", target_bir_lowering=False, debug=False,
                   num_devices=NCORES)

    adjt_fe = nc.dram_tensor("adjt_fe", [128, MCH * NFC], BF16,
                             kind="ExternalInput")
    adjt_ef = nc.dram_tensor("adjt_ef", [128, NCH * NEC], BF16,
                             kind="ExternalInput")
    f0t = nc.dram_tensor("f0t", [B, 128, N_F], BF16, kind="ExternalInput")
    e0t = nc.dram_tensor("e0t", [B, 128, N_E], BF16, kind="ExternalInput")
    f0t_own = nc.dram_tensor("f0t_own", [B, 128, NFC], BF16,
                             kind="ExternalInput")
    e0t_own = nc.dram_tensor("e0t_own", [B, 128, NEC], BF16,
                             kind="ExternalInput")
    # packed weights: [0:64, h*128:+128]=wft[h]; [64:128, same]=wet[h];
    # cols 512+h*64: Wf[h]; cols 768+h*64: We[h]; col 1024+h: a_fe (rows
    # 0:64) / a_ef (rows 64:128)
    wpack = nc.dram_tensor("wpack", [128, 1032], F32, kind="ExternalInput")
    out_fe = nc.dram_tensor("out_fe", [B, NFC, HEADS * HDIM], F32,
                            kind="ExternalOutput")
    out_ef = nc.dram_tensor("out_ef", [B, NEC, HEADS * HDIM], F32,
                            kind="ExternalOutput")

    with tile.TileContext(nc) as tc:
        with (
            tc.tile_pool(name="persist", bufs=1) as pp,
            tc.tile_pool(name="work", bufs=4) as wp,
            tc.tile_pool(name="pmain", bufs=3) as pmain,
            tc.tile_pool(name="psum", bufs=1,
                         space=bass.MemorySpace.PSUM) as ps,
        ):
            # PSUM bank budget (8 banks): proj 2 + rowtmp 1 + pbc 1 +
            # pofe 2 + poef 2.
            # ---------- stage 0: weights & attention vectors ----------
            adjfe_s = pp.tile([128, MCH * NFC], BF16, tag="adjfe")
            adjef_s = pp.tile([128, NCH * NEC], BF16, tag="adjef")

            wpack_s = pp.tile([128, 1032], F32, tag="wpack")
            nc.sync.dma_start(wpack_s[:], wpack[:])
            rhsF = pp.tile([128, BW], BF16, tag="rhsF")  # [Wf | Wf@a_ef] x4
            rhsE = pp.tile([128, BW], BF16, tag="rhsE")  # [We | We@a_fe] x4
            wsfe = pp.tile([128, HEADS], BF16, tag="wsfe")  # Wf@a_fe
            wsef = pp.tile([128, HEADS], BF16, tag="wsef")  # We@a_ef
            def emit_wchain(h):
                wfth = wpack_s[0:HDIM, h * 128:(h + 1) * 128]
                weth = wpack_s[HDIM:128, h * 128:(h + 1) * 128]
                afeh = wpack_s[0:HDIM, 1024 + h:1025 + h]
                aefh = wpack_s[0:HDIM, 1028 + h:1029 + h]
                afeh2 = wpack_s[HDIM:128, 1024 + h:1025 + h]
                aefh2 = wpack_s[HDIM:128, 1028 + h:1029 + h]
                nc.scalar.copy(rhsF[:, h * W:h * W + HDIM],
                               wpack_s[:, 512 + h * HDIM:512 + (h + 1) * HDIM])
                nc.scalar.copy(rhsE[:, h * W:h * W + HDIM],
                               wpack_s[:, 768 + h * HDIM:768 + (h + 1) * HDIM])
                pw = ps.tile([128, 1], F32, tag="proj", name="pw", bufs=2)
                nc.tensor.matmul(pw[:], wfth, aefh, start=True, stop=True)
                nc.scalar.copy(rhsF[:, h * W + HDIM:h * W + W], pw[:])
                pw = ps.tile([128, 1], F32, tag="proj", name="pw", bufs=2)
                nc.tensor.matmul(pw[:], wfth, afeh, start=True, stop=True)
                nc.scalar.copy(wsfe[:, h:h + 1], pw[:])
                pw = ps.tile([128, 1], F32, tag="proj", name="pw", bufs=2)
                nc.tensor.matmul(pw[:], weth, afeh2, start=True, stop=True)
                nc.scalar.copy(rhsE[:, h * W + HDIM:h * W + W], pw[:])
                pw = ps.tile([128, 1], F32, tag="proj", name="pw", bufs=2)
                nc.tensor.matmul(pw[:], weth, aefh2, start=True, stop=True)
                nc.scalar.copy(wsef[:, h:h + 1], pw[:])

            ones_bc = pp.tile([1, 128], BF16, tag="ones_bc")
            nc.vector.memset(ones_bc[:], 1.0)

            # ---------- es8 broadcast tiles (tiny PE work, emitted early) --
            f0own_s = [pp.tile([128, NFC], BF16, tag=f"f0o{b}", name=f"f0o{b}")
                       for b in range(B)]
            e0own_s = [pp.tile([128, NEC], BF16, tag=f"e0o{b}", name=f"e0o{b}")
                       for b in range(B)]
            for b in range(B):
                nc.sync.dma_start(f0own_s[b][:], f0t_own[b])
                nc.sync.dma_start(e0own_s[b][:], e0t_own[b])

            es8b_fe = [[pp.tile([128, NFC], BF16, tag=f"es8fe{b}{h}",
                                name=f"es8fe{b}{h}") for h in range(HEADS)]
                       for b in range(B)]
            es8b_ef = [[pp.tile([128, NEC], BF16, tag=f"es8ef{b}{h}",
                                name=f"es8ef{b}{h}") for h in range(HEADS)]
                       for b in range(B)]
            def emit_es8b_h(b, h):
                if True:
                    srow = ps.tile([1, NFC], F32, tag="proj", name="srow",
                                   bufs=2)
                    nc.tensor.matmul(srow[:], wsfe[:, h:h + 1], f0own_s[b][:],
                                     start=True, stop=True)
                    erow = wp.tile([1, NFC], BF16, tag="erow", name="erow")
                    nc.scalar.activation(erow[:], srow[:], AF.Exp, scale=0.8)
                    pbc = ps.tile([128, NFC], F32, tag="proj", name="pbc",
                                  bufs=2)
                    nc.tensor.matmul(pbc[:], ones_bc[:], erow[:],
                                     start=True, stop=True)
                    if b == 0:
                        nc.vector.tensor_copy(es8b_fe[b][h][:], pbc[:])
                    else:
                        nc.scalar.copy(es8b_fe[b][h][:], pbc[:])
                    srow = ps.tile([1, NEC], F32, tag="proj", name="srow",
                                   bufs=2)
                    nc.tensor.matmul(srow[:], wsef[:, h:h + 1], e0own_s[b][:],
                                     start=True, stop=True)
                    erow = wp.tile([1, NEC], BF16, tag="erow", name="erow")
                    nc.scalar.activation(erow[:], srow[:], AF.Exp, scale=0.8)
                    pbc = ps.tile([128, NEC], F32, tag="proj", name="pbc",
                                  bufs=2)
                    nc.tensor.matmul(pbc[:], ones_bc[:], erow[:],
                                     start=True, stop=True)
                    if b == 0:
                        nc.vector.tensor_copy(es8b_ef[b][h][:], pbc[:])
                    else:
                        nc.scalar.copy(es8b_ef[b][h][:], pbc[:])

            # ---------- persistent per-batch tensors ----------
            e1s = [pp.tile([128, MCH, BW], BF16, tag=f"e1s{b}", name=f"e1s{b}")
                   for b in range(B)]
            f1s = [pp.tile([128, NCH, BW], BF16, tag=f"f1s{b}", name=f"f1s{b}")
                   for b in range(B)]
            # t columns and their exps, split in quarter-chunk tiles so the
            # main loop can start after the first quarter of the projections.
            NG = 4
            MH, NH = MCH // NG, NCH // NG
            tfeb = [[pp.tile([128, MH, HEADS], F32, tag=f"tfeb{b}{g}",
                             name=f"tfeb{b}{g}") for g in range(NG)]
                    for b in range(B)]
            tefb = [[pp.tile([128, NH, HEADS], F32, tag=f"tefb{b}{g}",
                             name=f"tefb{b}{g}") for g in range(NG)]
                    for b in range(B)]
            et1_fe = [[[pp.tile([128, MH], F32, tag=f"et1fe{b}{h}{g}",
                                name=f"et1fe{b}{h}{g}") for g in range(NG)]
                       for h in range(HEADS)] for b in range(B)]
            et2_fe = [[[pp.tile([128, MH], F32, tag=f"et2fe{b}{h}{g}",
                                name=f"et2fe{b}{h}{g}") for g in range(NG)]
                       for h in range(HEADS)] for b in range(B)]
            et1_ef = [[[pp.tile([128, NH], F32, tag=f"et1ef{b}{h}{g}",
                                name=f"et1ef{b}{h}{g}") for g in range(NG)]
                       for h in range(HEADS)] for b in range(B)]
            et2_ef = [[[pp.tile([128, NH], F32, tag=f"et2ef{b}{h}{g}",
                                name=f"et2ef{b}{h}{g}") for g in range(NG)]
                       for h in range(HEADS)] for b in range(B)]
            ofe_stage = [[pp.tile([128, HEADS * HDIM], F32, tag=f"ofe{b}{i}",
                                  name=f"ofe{b}{i}") for i in range(NI_FE)]
                         for b in range(B)]
            oef_stage = [[pp.tile([128, HEADS * HDIM], F32, tag=f"oef{b}{i}",
                                  name=f"oef{b}{i}") for i in range(NI_EF)]
                         for b in range(B)]

            e0t_s = [pp.tile([128, N_E], BF16, tag=f"e0t{b}", name=f"e0t{b}")
                     for b in range(B)]
            f0t_s = [pp.tile([128, N_F], BF16, tag=f"f0t{b}", name=f"f0t{b}")
                     for b in range(B)]

            def proj_tile(b, ct, e_side):
                if e_side:
                    src, bundle, dst, tcols, n_half = (
                        e0t_s[b], rhsE, e1s, tfeb, MCH // 4)
                else:
                    src, bundle, dst, tcols, n_half = (
                        f0t_s[b], rhsF, f1s, tefb, NCH // 4)
                p_t = ps.tile([128, BW], F32, tag="proj", name="proj", bufs=2)
                nc.tensor.matmul(p_t[:], src[:, ct * 128:(ct + 1) * 128],
                                 bundle[:], start=True, stop=True)
                # b=0 front is ACT-bound while DVE idles: alternate the
                # PSUM->SBUF copies between the two engines
                tc_view = p_t[:].rearrange("p (h w) -> p h w", w=W)[:, :, HDIM]
                if b == 0 and e_side and ct % 2 == 1:
                    nc.vector.tensor_copy(
                        tcols[b][ct // n_half][:, ct % n_half, :], tc_view)
                    nc.vector.tensor_copy(dst[b][:, ct, :], p_t[:])
                else:
                    nc.scalar.copy(
                        tcols[b][ct // n_half][:, ct % n_half, :], tc_view)
                    nc.scalar.copy(dst[b][:, ct, :], p_t[:])

            def emit_exps(b, g, fe_side):
                tc_t, o1, o2 = ((tfeb, et1_fe, et2_fe) if fe_side
                                else (tefb, et1_ef, et2_ef))
                for h in range(HEADS):
                    nc.scalar.activation(o1[b][h][g][:], tc_t[b][g][:, :, h],
                                         AF.Exp, scale=1.0)
                    nc.scalar.activation(o2[b][h][g][:], tc_t[b][g][:, :, h],
                                         AF.Exp, scale=0.2)

            def emit_projections(b):
                nc.sync.dma_start(e0t_s[b][:], e0t[b])
                if b == 0:
                    # adjacency halves queued right after the E projections
                    # input; TT consumes them half by half
                    half = MCH * NFC // 2
                    nc.sync.dma_start(adjfe_s[:, 0:half], adjt_fe[:, 0:half])
                    nc.sync.dma_start(adjfe_s[:, half:], adjt_fe[:, half:])
                nc.sync.dma_start(f0t_s[b][:], f0t[b])
                if b == 0:
                    half = NCH * NEC // 2
                    nc.sync.dma_start(adjef_s[:, 0:half], adjt_ef[:, 0:half])
                    nc.sync.dma_start(adjef_s[:, half:], adjt_ef[:, half:])
                for g in range(NG):
                    for mt in range(g * MH, (g + 1) * MH):
                        proj_tile(b, mt, True)
                    emit_exps(b, g, True)
                for g in range(NG):
                    for nt in range(g * NH, (g + 1) * NH):
                        proj_tile(b, nt, False)
                    emit_exps(b, g, False)
                # ones columns for the denominator (Pool engine: off the
                # DVE critical path)
                nc.gpsimd.memset(
                    e1s[b][:].rearrange("p c (h w) -> p c h w", w=W)
                    [:, :, :, HDIM], 1.0)
                nc.gpsimd.memset(
                    f1s[b][:].rearrange("p c (h w) -> p c h w", w=W)
                    [:, :, :, HDIM], 1.0)

            def emit_ts_tt(b, h, fe_side):
                if fe_side:
                    nch, ncols, es8b, e1, e2, adj, half = (
                        MCH, NFC, es8b_fe, et1_fe, et2_fe, adjfe_s, MCH // 4)
                else:
                    nch, ncols, es8b, e1, e2, adj, half = (
                        NCH, NEC, es8b_ef, et1_ef, et2_ef, adjef_s, NCH // 4)
                pt = pmain.tile([128, nch * ncols], BF16, tag="P", name="P")
                hf = nch * ncols // 2
                for c in range(nch):
                    g, cg = divmod(c, half)
                    nc.vector.tensor_scalar(
                        pt[:, c * ncols:(c + 1) * ncols], es8b[b][h][:],
                        e1[b][h][g][:, cg:cg + 1], e2[b][h][g][:, cg:cg + 1],
                        ALU.mult, ALU.max)
                nc.vector.tensor_mul(pt[:, 0:hf], pt[:, 0:hf], adj[:, 0:hf])
                nc.vector.tensor_mul(pt[:, hf:], pt[:, hf:], adj[:, hf:])
                return pt

            def emit_matmuls(b, h, pt, fe_side):
                if fe_side:
                    nch, ncols, ni, x1s, potag = MCH, NFC, NI_FE, e1s, "pofe"
                else:
                    nch, ncols, ni, x1s, potag = NCH, NEC, NI_EF, f1s, "poef"
                po = ps.tile([128, ni, W], F32, tag=potag, name=potag, bufs=2)
                for i in range(ni):
                    for c in range(nch):
                        nc.tensor.matmul(
                            po[:, i, :],
                            pt[:, c * ncols + i * 128:c * ncols + (i + 1) * 128],
                            x1s[b][:, c, h * W:(h + 1) * W],
                            start=(c == 0), stop=(c == nch - 1),
                            skip_group_check=True)
                return po

            def emit_finalize(b, h, po, fe_side):
                ni, stage = ((NI_FE, ofe_stage) if fe_side
                             else (NI_EF, oef_stage))
                rec = wp.tile([128, ni], F32, tag="rec", name="rec")
                nc.vector.reciprocal(rec[:], po[:, :, HDIM])
                for i in range(ni):
                    nc.scalar.activation(
                        stage[b][i][:, h * HDIM:(h + 1) * HDIM],
                        po[:, i, 0:HDIM], AF.Copy, scale=rec[:, i:i + 1])

            def emit_stage4(b, fe_side):
                if not fe_side:
                    for i in range(NI_EF):
                        fin = wp.tile([128, HEADS * HDIM], F32, tag="fin",
                                      name="fin")
                        nc.scalar.activation(fin[:], oef_stage[b][i][:],
                                             AF.Relu)
                        nc.sync.dma_start(out_ef[b, i * 128:(i + 1) * 128, :],
                                          fin[:])
                    return
                for i in range(NI_FE):
                    x = ofe_stage[b][i]
                    mx = wp.tile([128, 1], F32, tag="mx", name="mx")
                    nc.vector.tensor_reduce(mx[:], x[:], mybir.AxisListType.X,
                                            ALU.max, negate=True)
                    ex = wp.tile([128, HEADS * HDIM], F32, tag="ex", name="ex")
                    sm = wp.tile([128, 1], F32, tag="sm", name="sm")
                    nc.scalar.activation(ex[:], x[:], AF.Exp, bias=mx[:],
                                         scale=1.0, accum_out=sm[:])
                    rc = wp.tile([128, 1], F32, tag="rc", name="rc")
                    nc.vector.reciprocal(rc[:], sm[:])
                    fin = wp.tile([128, HEADS * HDIM], F32, tag="fin",
                                  name="fin")
                    nc.scalar.activation(fin[:], ex[:], AF.Copy, scale=rc[:])
                    nc.sync.dma_start(out_fe[b, i * 128:(i + 1) * 128, :],
                                      fin[:])

            # ---------- main pipeline ----------
            # Phase order: proj(0), fe(0), proj(1), ef(0), fe(1), ef(1).
            # Within a phase, each head's finalize (PSUM reads) is deferred
            # one step so DVE never stalls waiting on PE accumulation.
            pending = []  # deferred finalize closures

            def pop_pending():
                while pending:
                    pending.pop(0)()

            def phase(b, fe_side):
                for h in range(HEADS):
                    pt = emit_ts_tt(b, h, fe_side)
                    pop_pending()
                    po = emit_matmuls(b, h, pt, fe_side)

                    def fin(b=b, h=h, po=po, fe=fe_side):
                        emit_finalize(b, h, po, fe)
                        if h == HEADS - 1:
                            emit_stage4(b, fe)
                    pending.append(fin)

            for h in range(HEADS):
                emit_wchain(h)
                emit_es8b_h(0, h)
            emit_projections(0)
            for h in range(HEADS):
                emit_es8b_h(1, h)
            phase(0, True)
            emit_projections(1)
            phase(0, False)
            phase(1, True)
            phase(1, False)
            pop_pending()

    nc.compile()
    return nc


_NC_CACHE = None


def _get_program():
    global _NC_CACHE
    if _NC_CACHE is None:
        _NC_CACHE = build_program()
    return _NC_CACHE


def _pack_adj(adj_t_slice, nchunks, ncols):
    # [nchunks*128, ncols] -> [128, nchunks*ncols] with chunk-major free dim
    a = np.ascontiguousarray(adj_t_slice.astype(ml_dtypes.bfloat16))
    a = a.reshape(nchunks, 128, ncols).transpose(1, 0, 2)
    return np.ascontiguousarray(a.reshape(128, nchunks * ncols))


def make_in_maps(F0, E0, adj_fe, adj_ef, Wf, We, a_fe, a_ef):
    F0 = np.asarray(F0, dtype=np.float32)
    E0 = np.asarray(E0, dtype=np.float32)
    Wf = np.asarray(Wf, dtype=np.float32)
    We = np.asarray(We, dtype=np.float32)
    a_fe = np.asarray(a_fe, dtype=np.float32)
    a_ef = np.asarray(a_ef, dtype=np.float32)

    f0t = np.ascontiguousarray(F0.transpose(0, 2, 1)).astype(ml_dtypes.bfloat16)
    e0t = np.ascontiguousarray(E0.transpose(0, 2, 1)).astype(ml_dtypes.bfloat16)
    adjt_fe_full = np.asarray(adj_fe).T.astype(np.float32)  # [N_E, N_F]
    adjt_ef_full = np.asarray(adj_ef).T.astype(np.float32)  # [N_F, N_E]
    wpack = np.zeros((128, 1032), np.float32)
    for h in range(HEADS):
        wpack[0:HDIM, h * 128:(h + 1) * 128] = Wf[h].T
        wpack[HDIM:128, h * 128:(h + 1) * 128] = We[h].T
        wpack[:, 512 + h * HDIM:512 + (h + 1) * HDIM] = Wf[h]
        wpack[:, 768 + h * HDIM:768 + (h + 1) * HDIM] = We[h]
        wpack[0:HDIM, 1024 + h] = a_fe[h]
        wpack[HDIM:128, 1024 + h] = a_fe[h]
        wpack[0:HDIM, 1028 + h] = a_ef[h]
        wpack[HDIM:128, 1028 + h] = a_ef[h]

    in_maps = []
    for k in range(NCORES):
        nsl = slice(k * NFC, (k + 1) * NFC)
        msl = slice(k * NEC, (k + 1) * NEC)
        in_maps.append({
            "adjt_fe": _pack_adj(adjt_fe_full[:, nsl], MCH, NFC),
            "adjt_ef": _pack_adj(adjt_ef_full[:, msl], NCH, NEC),
            "f0t": f0t,
            "e0t": e0t,
            "f0t_own": np.ascontiguousarray(f0t[:, :, nsl]),
            "e0t_own": np.ascontiguousarray(e0t[:, :, msl]),
            "wpack": wpack,
        })
    return in_maps


def kernel(F0, E0, adj_fe, adj_ef, Wf, We, a_fe, a_ef):
    nc = _get_program()
    in_maps = make_in_maps(F0, E0, adj_fe, adj_ef, Wf, We, a_fe, a_ef)
    res = run_bass_kernel_spmd(nc, in_maps, list(range(NCORES)))
    out_fe = np.concatenate([res.results[k]["out_fe"] for k in range(NCORES)],
                            axis=1)
    out_ef = np.concatenate([res.results[k]["out_ef"] for k in range(NCORES)],
                            axis=1)
    return (np.asarray(out_fe, dtype=np.float32),
            np.asarray(out_ef, dtype=np.float32))


# revision 29
# speedup vs baseline: 1.0469x; 1.0469x over previous
"""Bipartite graph multi-head attention (GAT-style) on 8 TRN2 NeuronCores.

Math (per batch b, head h, direction fe shown; ef symmetric):
    F1 = F0 @ Wf[h]; E1 = E0 @ We[h]
    s[n] = F1[n]·a;  t[m] = E1[m]·a
    u = s[n] + t[m];  e = leaky_relu(u, 0.2) masked by adj
    alpha = softmax_m(e);  out = alpha @ E1

Key identity: exp(lrelu(u)) = exp(0.2u)·max(exp(0.8u), 1); the row
factor exp(0.2 s[n]) cancels in the softmax ratio, so the unnormalized
weight (transposed, contraction dim on partitions) is
    G[m,n] = adjT[m,n] · max(es8[n]·exp(t[m]), exp(0.2·t[m]))
with es8 = exp(0.8 s).  One tensor_scalar (two fused ALU ops with
per-partition scalars) + one tensor_tensor against the adjacency per
score element; numerator and denominator come from a PE matmul against
[E1 | 1]; out = numer / denom.

Sharding: fe output rows n (2048) and ef output rows m (4096) split 8
ways; adjacency slices are the only large per-core inputs; projections
are computed replicated (no collectives).
"""

import os
import sys

import numpy as np

if "/opt/trn_rl_repo" not in sys.path:
    sys.path.insert(0, "/opt/trn_rl_repo")

import ml_dtypes

import concourse.bass as bass
import concourse.tile as tile
from concourse import bacc, mybir
from concourse.bass_utils import run_bass_kernel_spmd

BF16 = mybir.dt.bfloat16
F32 = mybir.dt.float32
AF = mybir.ActivationFunctionType
ALU = mybir.AluOpType

B, N_F, N_E = 2, 2048, 4096
FDIM, HDIM, HEADS = 128, 64, 4
NCORES = 8
NFC = N_F // NCORES  # 256 fe rows per core
NEC = N_E // NCORES  # 512 ef rows per core
MCH = N_E // 128  # 32 m-chunks (fe contraction)
NCH = N_F // 128  # 16 n-chunks (ef contraction)
W = HDIM + 1  # 65: [dst feature | ones] columns
BW = HEADS * W  # 260: bundled projection width
NI_FE = NFC // 128  # 2 output row tiles (fe)
NI_EF = NEC // 128  # 4 output row tiles (ef)


def build_program():
    nc = bacc.Bacc("TRN2", target_bir_lowering=False, debug=False,
                   num_devices=NCORES)

    adjt_fe = nc.dram_tensor("adjt_fe", [128, MCH * NFC], BF16,
                             kind="ExternalInput")
    adjt_ef = nc.dram_tensor("adjt_ef", [128, NCH * NEC], BF16,
                             kind="ExternalInput")
    f0t = nc.dram_tensor("f0t", [B, 128, N_F], BF16, kind="ExternalInput")
    e0t = nc.dram_tensor("e0t", [B, 128, N_E], BF16, kind="ExternalInput")
    f0t_own = nc.dram_tensor("f0t_own", [B, 128, NFC], BF16,
                             kind="ExternalInput")
    e0t_own = nc.dram_tensor("e0t_own", [B, 128, NEC], BF16,
                             kind="ExternalInput")
    # packed weights: [0:64, h*128:+128]=wft[h]; [64:128, same]=wet[h];
    # cols 512+h*64: Wf[h]; cols 768+h*64: We[h]; col 1024+h: a_fe (rows
    # 0:64) / a_ef (rows 64:128)
    wpack = nc.dram_tensor("wpack", [128, 1032], F32, kind="ExternalInput")
    out_fe = nc.dram_tensor("out_fe", [B, NFC, HEADS * HDIM], F32,
                            kind="ExternalOutput")
    out_ef = nc.dram_tensor("out_ef", [B, NEC, HEADS * HDIM], F32,
                            kind="ExternalOutput")

    with tile.TileContext(nc) as tc:
        with (
            tc.tile_pool(name="persist", bufs=1) as pp,
            tc.tile_pool(name="work", bufs=4) as wp,
            tc.tile_pool(name="pmain", bufs=3) as pmain,
            tc.tile_pool(name="psum", bufs=1,
                         space=bass.MemorySpace.PSUM) as ps,
        ):
            # PSUM bank budget (8 banks): proj 2 + rowtmp 1 + pbc 1 +
            # pofe 2 + poef 2.
            # ---------- stage 0: weights & attention vectors ----------
            adjfe_s = pp.tile([128, MCH * NFC], BF16, tag="adjfe")
            adjef_s = pp.tile([128, NCH * NEC], BF16, tag="adjef")

            wpack_s = pp.tile([128, 1032], F32, tag="wpack")
            nc.sync.dma_start(wpack_s[:], wpack[:])
            rhsF = pp.tile([128, BW], BF16, tag="rhsF")  # [Wf | Wf@a_ef] x4
            rhsE = pp.tile([128, BW], BF16, tag="rhsE")  # [We | We@a_fe] x4
            wsfe = pp.tile([128, HEADS], BF16, tag="wsfe")  # Wf@a_fe
            wsef = pp.tile([128, HEADS], BF16, tag="wsef")  # We@a_ef
            def emit_wchain(h):
                wfth = wpack_s[0:HDIM, h * 128:(h + 1) * 128]
                weth = wpack_s[HDIM:128, h * 128:(h + 1) * 128]
                afeh = wpack_s[0:HDIM, 1024 + h:1025 + h]
                aefh = wpack_s[0:HDIM, 1028 + h:1029 + h]
                afeh2 = wpack_s[HDIM:128, 1024 + h:1025 + h]
                aefh2 = wpack_s[HDIM:128, 1028 + h:1029 + h]
                nc.scalar.copy(rhsF[:, h * W:h * W + HDIM],
                               wpack_s[:, 512 + h * HDIM:512 + (h + 1) * HDIM])
                nc.scalar.copy(rhsE[:, h * W:h * W + HDIM],
                               wpack_s[:, 768 + h * HDIM:768 + (h + 1) * HDIM])
                pw = ps.tile([128, 1], F32, tag="proj", name="pw", bufs=2)
                nc.tensor.matmul(pw[:], wfth, aefh, start=True, stop=True)
                nc.scalar.copy(rhsF[:, h * W + HDIM:h * W + W], pw[:])
                pw = ps.tile([128, 1], F32, tag="proj", name="pw", bufs=2)
                nc.tensor.matmul(pw[:], wfth, afeh, start=True, stop=True)
                nc.scalar.copy(wsfe[:, h:h + 1], pw[:])
                pw = ps.tile([128, 1], F32, tag="proj", name="pw", bufs=2)
                nc.tensor.matmul(pw[:], weth, afeh2, start=True, stop=True)
                nc.scalar.copy(rhsE[:, h * W + HDIM:h * W + W], pw[:])
                pw = ps.tile([128, 1], F32, tag="proj", name="pw", bufs=2)
                nc.tensor.matmul(pw[:], weth, aefh2, start=True, stop=True)
                nc.scalar.copy(wsef[:, h:h + 1], pw[:])

            ones_bc = pp.tile([1, 128], BF16, tag="ones_bc")
            nc.vector.memset(ones_bc[:], 1.0)

            # ---------- es8 broadcast tiles (tiny PE work, emitted early) --
            f0own_s = [pp.tile([128, NFC], BF16, tag=f"f0o{b}", name=f"f0o{b}")
                       for b in range(B)]
            e0own_s = [pp.tile([128, NEC], BF16, tag=f"e0o{b}", name=f"e0o{b}")
                       for b in range(B)]
            for b in range(B):
                nc.sync.dma_start(f0own_s[b][:], f0t_own[b])
                nc.sync.dma_start(e0own_s[b][:], e0t_own[b])

            es8b_fe = [[pp.tile([128, NFC], BF16, tag=f"es8fe{b}{h}",
                                name=f"es8fe{b}{h}") for h in range(HEADS)]
                       for b in range(B)]
            es8b_ef = [[pp.tile([128, NEC], BF16, tag=f"es8ef{b}{h}",
                                name=f"es8ef{b}{h}") for h in range(HEADS)]
                       for b in range(B)]
            def emit_es8b_h(b, h):
                # b=0 chains use the (front-idle) po banks so they don't
                # contend with projection PSUM slots; b=1 chains use proj
                stag, ptag = ("pofe", "poef") if b == 0 else ("proj", "proj")
                if True:
                    srow = ps.tile([1, NFC], F32, tag=stag, name="srow",
                                   bufs=2)
                    nc.tensor.matmul(srow[:], wsfe[:, h:h + 1], f0own_s[b][:],
                                     start=True, stop=True)
                    erow = wp.tile([1, NFC], BF16, tag="erow", name="erow")
                    nc.scalar.activation(erow[:], srow[:], AF.Exp, scale=0.8)
                    pbc = ps.tile([128, NFC], F32, tag=ptag, name="pbc",
                                  bufs=2)
                    nc.tensor.matmul(pbc[:], ones_bc[:], erow[:],
                                     start=True, stop=True)
                    if b == 0:
                        nc.vector.tensor_copy(es8b_fe[b][h][:], pbc[:])
                    else:
                        nc.scalar.copy(es8b_fe[b][h][:], pbc[:])
                    srow = ps.tile([1, NEC], F32, tag=stag, name="srow",
                                   bufs=2)
                    nc.tensor.matmul(srow[:], wsef[:, h:h + 1], e0own_s[b][:],
                                     start=True, stop=True)
                    erow = wp.tile([1, NEC], BF16, tag="erow", name="erow")
                    nc.scalar.activation(erow[:], srow[:], AF.Exp, scale=0.8)
                    pbc = ps.tile([128, NEC], F32, tag=ptag, name="pbc",
                                  bufs=2)
                    nc.tensor.matmul(pbc[:], ones_bc[:], erow[:],
                                     start=True, stop=True)
                    if b == 0:
                        nc.vector.tensor_copy(es8b_ef[b][h][:], pbc[:])
                    else:
                        nc.scalar.copy(es8b_ef[b][h][:], pbc[:])

            # ---------- persistent per-batch tensors ----------
            e1s = [pp.tile([128, MCH, BW], BF16, tag=f"e1s{b}", name=f"e1s{b}")
                   for b in range(B)]
            f1s = [pp.tile([128, NCH, BW], BF16, tag=f"f1s{b}", name=f"f1s{b}")
                   for b in range(B)]
            # t columns and their exps, split in quarter-chunk tiles so the
            # main loop can start after the first quarter of the projections.
            NG = 4
            MH, NH = MCH // NG, NCH // NG
            tfeb = [[pp.tile([128, MH, HEADS], F32, tag=f"tfeb{b}{g}",
                             name=f"tfeb{b}{g}") for g in range(NG)]
                    for b in range(B)]
            tefb = [[pp.tile([128, NH, HEADS], F32, tag=f"tefb{b}{g}",
                             name=f"tefb{b}{g}") for g in range(NG)]
                    for b in range(B)]
            et1_fe = [[[pp.tile([128, MH], F32, tag=f"et1fe{b}{h}{g}",
                                name=f"et1fe{b}{h}{g}") for g in range(NG)]
                       for h in range(HEADS)] for b in range(B)]
            et2_fe = [[[pp.tile([128, MH], F32, tag=f"et2fe{b}{h}{g}",
                                name=f"et2fe{b}{h}{g}") for g in range(NG)]
                       for h in range(HEADS)] for b in range(B)]
            et1_ef = [[[pp.tile([128, NH], F32, tag=f"et1ef{b}{h}{g}",
                                name=f"et1ef{b}{h}{g}") for g in range(NG)]
                       for h in range(HEADS)] for b in range(B)]
            et2_ef = [[[pp.tile([128, NH], F32, tag=f"et2ef{b}{h}{g}",
                                name=f"et2ef{b}{h}{g}") for g in range(NG)]
                       for h in range(HEADS)] for b in range(B)]
            ofe_stage = [[pp.tile([128, HEADS * HDIM], F32, tag=f"ofe{b}{i}",
                                  name=f"ofe{b}{i}") for i in range(NI_FE)]
                         for b in range(B)]
            oef_stage = [[pp.tile([128, HEADS * HDIM], F32, tag=f"oef{b}{i}",
                                  name=f"oef{b}{i}") for i in range(NI_EF)]
                         for b in range(B)]

            e0t_s = [pp.tile([128, N_E], BF16, tag=f"e0t{b}", name=f"e0t{b}")
                     for b in range(B)]
            f0t_s = [pp.tile([128, N_F], BF16, tag=f"f0t{b}", name=f"f0t{b}")
                     for b in range(B)]

            def proj_tile(b, ct, e_side):
                if e_side:
                    src, bundle, dst, tcols, n_half = (
                        e0t_s[b], rhsE, e1s, tfeb, MCH // 4)
                else:
                    src, bundle, dst, tcols, n_half = (
                        f0t_s[b], rhsF, f1s, tefb, NCH // 4)
                p_t = ps.tile([128, BW], F32, tag="proj", name="proj", bufs=2)
                nc.tensor.matmul(p_t[:], src[:, ct * 128:(ct + 1) * 128],
                                 bundle[:], start=True, stop=True)
                # b=0 front is ACT-bound while DVE idles: alternate the
                # PSUM->SBUF copies between the two engines
                tc_view = p_t[:].rearrange("p (h w) -> p h w", w=W)[:, :, HDIM]
                if b == 0 and e_side and ct % 2 == 1:
                    nc.vector.tensor_copy(
                        tcols[b][ct // n_half][:, ct % n_half, :], tc_view)
                    nc.vector.tensor_copy(dst[b][:, ct, :], p_t[:])
                else:
                    nc.scalar.copy(
                        tcols[b][ct // n_half][:, ct % n_half, :], tc_view)
                    nc.scalar.copy(dst[b][:, ct, :], p_t[:])

            def emit_exps(b, g, fe_side):
                tc_t, o1, o2 = ((tfeb, et1_fe, et2_fe) if fe_side
                                else (tefb, et1_ef, et2_ef))
                for h in range(HEADS):
                    nc.scalar.activation(o1[b][h][g][:], tc_t[b][g][:, :, h],
                                         AF.Exp, scale=1.0)
                    nc.scalar.activation(o2[b][h][g][:], tc_t[b][g][:, :, h],
                                         AF.Exp, scale=0.2)

            def emit_projections(b):
                # quarter-split input loads: the first exp group (and so the
                # DVE main loop) unblocks after 1/4 of the transfer
                q = N_E // 4
                for i in range(4):
                    nc.sync.dma_start(e0t_s[b][:, i * q:(i + 1) * q],
                                      e0t[b, :, i * q:(i + 1) * q])
                if b == 0:
                    # adjacency halves queued right after the E projections
                    # input; TT consumes them half by half
                    half = MCH * NFC // 2
                    nc.sync.dma_start(adjfe_s[:, 0:half], adjt_fe[:, 0:half])
                    nc.sync.dma_start(adjfe_s[:, half:], adjt_fe[:, half:])
                q2 = N_F // 2
                for i in range(2):
                    nc.sync.dma_start(f0t_s[b][:, i * q2:(i + 1) * q2],
                                      f0t[b, :, i * q2:(i + 1) * q2])
                if b == 0:
                    half = NCH * NEC // 2
                    nc.sync.dma_start(adjef_s[:, 0:half], adjt_ef[:, 0:half])
                    nc.sync.dma_start(adjef_s[:, half:], adjt_ef[:, half:])
                for g in range(NG):
                    for mt in range(g * MH, (g + 1) * MH):
                        proj_tile(b, mt, True)
                    emit_exps(b, g, True)
                for g in range(NG):
                    for nt in range(g * NH, (g + 1) * NH):
                        proj_tile(b, nt, False)
                    emit_exps(b, g, False)
                # ones columns for the denominator (Pool engine: off the
                # DVE critical path)
                nc.gpsimd.memset(
                    e1s[b][:].rearrange("p c (h w) -> p c h w", w=W)
                    [:, :, :, HDIM], 1.0)
                nc.gpsimd.memset(
                    f1s[b][:].rearrange("p c (h w) -> p c h w", w=W)
                    [:, :, :, HDIM], 1.0)

            def emit_ts_tt(b, h, fe_side):
                if fe_side:
                    nch, ncols, es8b, e1, e2, adj, half = (
                        MCH, NFC, es8b_fe, et1_fe, et2_fe, adjfe_s, MCH // 4)
                else:
                    nch, ncols, es8b, e1, e2, adj, half = (
                        NCH, NEC, es8b_ef, et1_ef, et2_ef, adjef_s, NCH // 4)
                pt = pmain.tile([128, nch * ncols], BF16, tag="P", name="P")
                hf = nch * ncols // 2
                for c in range(nch):
                    g, cg = divmod(c, half)
                    nc.vector.tensor_scalar(
                        pt[:, c * ncols:(c + 1) * ncols], es8b[b][h][:],
                        e1[b][h][g][:, cg:cg + 1], e2[b][h][g][:, cg:cg + 1],
                        ALU.mult, ALU.max)
                nc.vector.tensor_mul(pt[:, 0:hf], pt[:, 0:hf], adj[:, 0:hf])
                nc.vector.tensor_mul(pt[:, hf:], pt[:, hf:], adj[:, hf:])
                return pt

            def emit_matmuls(b, h, pt, fe_side):
                if fe_side:
                    nch, ncols, ni, x1s, potag = MCH, NFC, NI_FE, e1s, "pofe"
                else:
                    nch, ncols, ni, x1s, potag = NCH, NEC, NI_EF, f1s, "poef"
                po = ps.tile([128, ni, W], F32, tag=potag, name=potag, bufs=2)
                for i in range(ni):
                    for c in range(nch):
                        nc.tensor.matmul(
                            po[:, i, :],
                            pt[:, c * ncols + i * 128:c * ncols + (i + 1) * 128],
                            x1s[b][:, c, h * W:(h + 1) * W],
                            start=(c == 0), stop=(c == nch - 1),
                            skip_group_check=True)
                return po

            def emit_finalize(b, h, po, fe_side):
                ni, stage = ((NI_FE, ofe_stage) if fe_side
                             else (NI_EF, oef_stage))
                rec = wp.tile([128, ni], F32, tag="rec", name="rec")
                nc.vector.reciprocal(rec[:], po[:, :, HDIM])
                for i in range(ni):
                    nc.scalar.activation(
                        stage[b][i][:, h * HDIM:(h + 1) * HDIM],
                        po[:, i, 0:HDIM], AF.Copy, scale=rec[:, i:i + 1])

            def emit_stage4(b, fe_side):
                if not fe_side:
                    for i in range(NI_EF):
                        fin = wp.tile([128, HEADS * HDIM], F32, tag="fin",
                                      name="fin")
                        nc.scalar.activation(fin[:], oef_stage[b][i][:],
                                             AF.Relu)
                        nc.sync.dma_start(out_ef[b, i * 128:(i + 1) * 128, :],
                                          fin[:])
                    return
                for i in range(NI_FE):
                    x = ofe_stage[b][i]
                    mx = wp.tile([128, 1], F32, tag="mx", name="mx")
                    nc.vector.tensor_reduce(mx[:], x[:], mybir.AxisListType.X,
                                            ALU.max, negate=True)
                    ex = wp.tile([128, HEADS * HDIM], F32, tag="ex", name="ex")
                    sm = wp.tile([128, 1], F32, tag="sm", name="sm")
                    nc.scalar.activation(ex[:], x[:], AF.Exp, bias=mx[:],
                                         scale=1.0, accum_out=sm[:])
                    rc = wp.tile([128, 1], F32, tag="rc", name="rc")
                    nc.vector.reciprocal(rc[:], sm[:])
                    fin = wp.tile([128, HEADS * HDIM], F32, tag="fin",
                                  name="fin")
                    nc.scalar.activation(fin[:], ex[:], AF.Copy, scale=rc[:])
                    nc.sync.dma_start(out_fe[b, i * 128:(i + 1) * 128, :],
                                      fin[:])

            # ---------- main pipeline ----------
            # Phase order: proj(0), fe(0), proj(1), ef(0), fe(1), ef(1).
            # Within a phase, each head's finalize (PSUM reads) is deferred
            # one step so DVE never stalls waiting on PE accumulation.
            pending = []  # deferred finalize closures

            def pop_pending():
                while pending:
                    pending.pop(0)()

            def phase(b, fe_side):
                last_phase = (b == B - 1) and not fe_side
                for h in range(HEADS):
                    pt = emit_ts_tt(b, h, fe_side)
                    pop_pending()
                    if last_phase and h == HEADS - 1:
                        # very last head: interleave finalizes with the
                        # accumulation groups to shorten the tail
                        po = ps.tile([128, NI_EF, W], F32, tag="poef",
                                     name="poef", bufs=2)
                        for i in range(NI_EF):
                            for c in range(NCH):
                                nc.tensor.matmul(
                                    po[:, i, :],
                                    pt[:, c * NEC + i * 128:
                                       c * NEC + (i + 1) * 128],
                                    f1s[b][:, c, h * W:(h + 1) * W],
                                    start=(c == 0), stop=(c == NCH - 1),
                                    skip_group_check=True)
                            rec = wp.tile([128, 1], F32, tag="rec",
                                          name="rec")
                            nc.vector.reciprocal(rec[:], po[:, i, HDIM:W])
                            nc.scalar.activation(
                                oef_stage[b][i][:, h * HDIM:(h + 1) * HDIM],
                                po[:, i, 0:HDIM], AF.Copy, scale=rec[:])
                        emit_stage4(b, False)
                        continue
                    po = emit_matmuls(b, h, pt, fe_side)

                    def fin(b=b, h=h, po=po, fe=fe_side):
                        emit_finalize(b, h, po, fe)
                        if h == HEADS - 1:
                            pending.append(
                                lambda b=b, fe=fe: emit_stage4(b, fe))
                    pending.append(fin)

            for h in range(HEADS):
                emit_wchain(h)
                emit_es8b_h(0, h)
            emit_projections(0)
            for h in range(HEADS):
                emit_es8b_h(1, h)
            phase(0, True)
            emit_projections(1)
            phase(0, False)
            phase(1, True)
            phase(1, False)
            pop_pending()

    nc.compile()
    return nc


_NC_CACHE = None


def _get_program():
    global _NC_CACHE
    if _NC_CACHE is None:
        _NC_CACHE = build_program()
    return _NC_CACHE


def _pack_adj(adj_t_slice, nchunks, ncols):
    # [nchunks*128, ncols] -> [128, nchunks*ncols] with chunk-major free dim
    a = np.ascontiguousarray(adj_t_slice.astype(ml_dtypes.bfloat16))
    a = a.reshape(nchunks, 128, ncols).transpose(1, 0, 2)
    return np.ascontiguousarray(a.reshape(128, nchunks * ncols))


def make_in_maps(F0, E0, adj_fe, adj_ef, Wf, We, a_fe, a_ef):
    F0 = np.asarray(F0, dtype=np.float32)
    E0 = np.asarray(E0, dtype=np.float32)
    Wf = np.asarray(Wf, dtype=np.float32)
    We = np.asarray(We, dtype=np.float32)
    a_fe = np.asarray(a_fe, dtype=np.float32)
    a_ef = np.asarray(a_ef, dtype=np.float32)

    f0t = np.ascontiguousarray(F0.transpose(0, 2, 1)).astype(ml_dtypes.bfloat16)
    e0t = np.ascontiguousarray(E0.transpose(0, 2, 1)).astype(ml_dtypes.bfloat16)
    adjt_fe_full = np.asarray(adj_fe).T.astype(np.float32)  # [N_E, N_F]
    adjt_ef_full = np.asarray(adj_ef).T.astype(np.float32)  # [N_F, N_E]
    wpack = np.zeros((128, 1032), np.float32)
    for h in range(HEADS):
        wpack[0:HDIM, h * 128:(h + 1) * 128] = Wf[h].T
        wpack[HDIM:128, h * 128:(h + 1) * 128] = We[h].T
        wpack[:, 512 + h * HDIM:512 + (h + 1) * HDIM] = Wf[h]
        wpack[:, 768 + h * HDIM:768 + (h + 1) * HDIM] = We[h]
        wpack[0:HDIM, 1024 + h] = a_fe[h]
        wpack[HDIM:128, 1024 + h] = a_fe[h]
        wpack[0:HDIM, 1028 + h] = a_ef[h]
        wpack[HDIM:128, 1028 + h] = a_ef[h]

    in_maps = []
    for k in range(NCORES):
        nsl = slice(k * NFC, (k + 1) * NFC)
        msl = slice(k * NEC, (k + 1) * NEC)
        in_maps.append({
            "adjt_fe": _pack_adj(adjt_fe_full[:, nsl], MCH, NFC),
            "adjt_ef": _pack_adj(adjt_ef_full[:, msl], NCH, NEC),
            "f0t": f0t,
            "e0t": e0t,
            "f0t_own": np.ascontiguousarray(f0t[:, :, nsl]),
            "e0t_own": np.ascontiguousarray(e0t[:, :, msl]),
            "wpack": wpack,
        })
    return in_maps


def kernel(F0, E0, adj_fe, adj_ef, Wf, We, a_fe, a_ef):
    nc = _get_program()
    in_maps = make_in_maps(F0, E0, adj_fe, adj_ef, Wf, We, a_fe, a_ef)
    res = run_bass_kernel_spmd(nc, in_maps, list(range(NCORES)))
    out_fe = np.concatenate([res.results[k]["out_fe"] for k in range(NCORES)],
                            axis=1)
    out_ef = np.concatenate([res.results[k]["out_ef"] for k in range(NCORES)],
                            axis=1)
    return (np.asarray(out_fe, dtype=np.float32),
            np.asarray(out_ef, dtype=np.float32))


# revision 32
# speedup vs baseline: 1.0568x; 1.0095x over previous
"""Bipartite graph multi-head attention (GAT-style) on 8 TRN2 NeuronCores.

Math (per batch b, head h, direction fe shown; ef symmetric):
    F1 = F0 @ Wf[h]; E1 = E0 @ We[h]
    s[n] = F1[n]·a;  t[m] = E1[m]·a
    u = s[n] + t[m];  e = leaky_relu(u, 0.2) masked by adj
    alpha = softmax_m(e);  out = alpha @ E1

Key identity: exp(lrelu(u)) = exp(0.2u)·max(exp(0.8u), 1); the row
factor exp(0.2 s[n]) cancels in the softmax ratio, so the unnormalized
weight (transposed, contraction dim on partitions) is
    G[m,n] = adjT[m,n] · max(es8[n]·exp(t[m]), exp(0.2·t[m]))
with es8 = exp(0.8 s).  One tensor_scalar (two fused ALU ops with
per-partition scalars) + one tensor_tensor against the adjacency per
score element; numerator and denominator come from a PE matmul against
[E1 | 1]; out = numer / denom.

Sharding: fe output rows n (2048) and ef output rows m (4096) split 8
ways; adjacency slices are the only large per-core inputs; projections
are computed replicated (no collectives).
"""

import os
import sys

import numpy as np

if "/opt/trn_rl_repo" not in sys.path:
    sys.path.insert(0, "/opt/trn_rl_repo")

import ml_dtypes

import concourse.bass as bass
import concourse.tile as tile
from concourse import bacc, mybir
from concourse.bass_utils import run_bass_kernel_spmd

BF16 = mybir.dt.bfloat16
F32 = mybir.dt.float32
AF = mybir.ActivationFunctionType
ALU = mybir.AluOpType

B, N_F, N_E = 2, 2048, 4096
FDIM, HDIM, HEADS = 128, 64, 4
NCORES = 8
NFC = N_F // NCORES  # 256 fe rows per core
NEC = N_E // NCORES  # 512 ef rows per core
MCH = N_E // 128  # 32 m-chunks (fe contraction)
NCH = N_F // 128  # 16 n-chunks (ef contraction)
W = HDIM + 1  # 65: [dst feature | ones] columns
BW = HEADS * W  # 260: bundled projection width
NI_FE = NFC // 128  # 2 output row tiles (fe)
NI_EF = NEC // 128  # 4 output row tiles (ef)


def build_program():
    nc = bacc.Bacc("TRN2", target_bir_lowering=False, debug=False,
                   num_devices=NCORES)

    adjt_fe = nc.dram_tensor("adjt_fe", [128, MCH * NFC], BF16,
                             kind="ExternalInput")
    adjt_ef = nc.dram_tensor("adjt_ef", [128, NCH * NEC], BF16,
                             kind="ExternalInput")
    f0t = nc.dram_tensor("f0t", [B, 128, N_F], BF16, kind="ExternalInput")
    e0t = nc.dram_tensor("e0t", [B, 128, N_E], BF16, kind="ExternalInput")
    f0t_own = nc.dram_tensor("f0t_own", [B, 128, NFC], BF16,
                             kind="ExternalInput")
    e0t_own = nc.dram_tensor("e0t_own", [B, 128, NEC], BF16,
                             kind="ExternalInput")
    # packed weights: [0:64, h*128:+128]=wft[h]; [64:128, same]=wet[h];
    # cols 512+h*64: Wf[h]; cols 768+h*64: We[h]; col 1024+h: a_fe (rows
    # 0:64) / a_ef (rows 64:128)
    wpack = nc.dram_tensor("wpack", [128, 1032], F32, kind="ExternalInput")
    out_fe = nc.dram_tensor("out_fe", [B, NFC, HEADS * HDIM], F32,
                            kind="ExternalOutput")
    out_ef = nc.dram_tensor("out_ef", [B, NEC, HEADS * HDIM], F32,
                            kind="ExternalOutput")

    with tile.TileContext(nc) as tc:
        with (
            tc.tile_pool(name="persist", bufs=1) as pp,
            tc.tile_pool(name="work", bufs=4) as wp,
            tc.tile_pool(name="pmain", bufs=3) as pmain,
            tc.tile_pool(name="psum", bufs=1,
                         space=bass.MemorySpace.PSUM) as ps,
        ):
            # PSUM bank budget (8 banks): proj 2 + rowtmp 1 + pbc 1 +
            # pofe 2 + poef 2.
            # ---------- stage 0: weights & attention vectors ----------
            adjfe_s = pp.tile([128, MCH * NFC], BF16, tag="adjfe")
            adjef_s = pp.tile([128, NCH * NEC], BF16, tag="adjef")

            wpack_s = pp.tile([128, 1032], F32, tag="wpack")
            nc.sync.dma_start(wpack_s[:], wpack[:])
            rhsF = pp.tile([128, BW], BF16, tag="rhsF")  # [Wf | Wf@a_ef] x4
            rhsE = pp.tile([128, BW], BF16, tag="rhsE")  # [We | We@a_fe] x4
            wsfe = pp.tile([128, HEADS], BF16, tag="wsfe")  # Wf@a_fe
            wsef = pp.tile([128, HEADS], BF16, tag="wsef")  # We@a_ef
            def emit_wchain(h):
                wfth = wpack_s[0:HDIM, h * 128:(h + 1) * 128]
                weth = wpack_s[HDIM:128, h * 128:(h + 1) * 128]
                afeh = wpack_s[0:HDIM, 1024 + h:1025 + h]
                aefh = wpack_s[0:HDIM, 1028 + h:1029 + h]
                afeh2 = wpack_s[HDIM:128, 1024 + h:1025 + h]
                aefh2 = wpack_s[HDIM:128, 1028 + h:1029 + h]
                nc.scalar.copy(rhsF[:, h * W:h * W + HDIM],
                               wpack_s[:, 512 + h * HDIM:512 + (h + 1) * HDIM])
                nc.scalar.copy(rhsE[:, h * W:h * W + HDIM],
                               wpack_s[:, 768 + h * HDIM:768 + (h + 1) * HDIM])
                pw = ps.tile([128, 1], F32, tag="proj", name="pw", bufs=2)
                nc.tensor.matmul(pw[:], wfth, aefh, start=True, stop=True)
                nc.scalar.copy(rhsF[:, h * W + HDIM:h * W + W], pw[:])
                pw = ps.tile([128, 1], F32, tag="proj", name="pw", bufs=2)
                nc.tensor.matmul(pw[:], wfth, afeh, start=True, stop=True)
                nc.scalar.copy(wsfe[:, h:h + 1], pw[:])
                pw = ps.tile([128, 1], F32, tag="proj", name="pw", bufs=2)
                nc.tensor.matmul(pw[:], weth, afeh2, start=True, stop=True)
                nc.scalar.copy(rhsE[:, h * W + HDIM:h * W + W], pw[:])
                pw = ps.tile([128, 1], F32, tag="proj", name="pw", bufs=2)
                nc.tensor.matmul(pw[:], weth, aefh2, start=True, stop=True)
                nc.scalar.copy(wsef[:, h:h + 1], pw[:])

            ones_bc = pp.tile([1, 128], BF16, tag="ones_bc")
            nc.vector.memset(ones_bc[:], 1.0)

            # ---------- es8 broadcast tiles (tiny PE work, emitted early) --
            f0own_s = [pp.tile([128, NFC], BF16, tag=f"f0o{b}", name=f"f0o{b}")
                       for b in range(B)]
            e0own_s = [pp.tile([128, NEC], BF16, tag=f"e0o{b}", name=f"e0o{b}")
                       for b in range(B)]
            for b in range(B):
                nc.sync.dma_start(f0own_s[b][:], f0t_own[b])
                nc.sync.dma_start(e0own_s[b][:], e0t_own[b])

            es8b_fe = [[pp.tile([128, NFC], BF16, tag=f"es8fe{b}{h}",
                                name=f"es8fe{b}{h}") for h in range(HEADS)]
                       for b in range(B)]
            es8b_ef = [[pp.tile([128, NEC], BF16, tag=f"es8ef{b}{h}",
                                name=f"es8ef{b}{h}") for h in range(HEADS)]
                       for b in range(B)]
            def emit_es8b_h(b, h):
                # b=0 chains use the (front-idle) po banks so they don't
                # contend with projection PSUM slots; b=1 chains use proj
                stag, ptag = ("pofe", "poef") if b == 0 else ("proj", "proj")
                if True:
                    srow = ps.tile([1, NFC], F32, tag=stag, name="srow",
                                   bufs=2)
                    nc.tensor.matmul(srow[:], wsfe[:, h:h + 1], f0own_s[b][:],
                                     start=True, stop=True)
                    erow = wp.tile([1, NFC], BF16, tag="erow", name="erow")
                    nc.scalar.activation(erow[:], srow[:], AF.Exp, scale=0.8)
                    pbc = ps.tile([128, NFC], F32, tag=ptag, name="pbc",
                                  bufs=2)
                    nc.tensor.matmul(pbc[:], ones_bc[:], erow[:],
                                     start=True, stop=True)
                    if b == 0:
                        nc.vector.tensor_copy(es8b_fe[b][h][:], pbc[:])
                    else:
                        nc.scalar.copy(es8b_fe[b][h][:], pbc[:])
                    srow = ps.tile([1, NEC], F32, tag=stag, name="srow",
                                   bufs=2)
                    nc.tensor.matmul(srow[:], wsef[:, h:h + 1], e0own_s[b][:],
                                     start=True, stop=True)
                    erow = wp.tile([1, NEC], BF16, tag="erow", name="erow")
                    nc.scalar.activation(erow[:], srow[:], AF.Exp, scale=0.8)
                    pbc = ps.tile([128, NEC], F32, tag=ptag, name="pbc",
                                  bufs=2)
                    nc.tensor.matmul(pbc[:], ones_bc[:], erow[:],
                                     start=True, stop=True)
                    if b == 0:
                        nc.vector.tensor_copy(es8b_ef[b][h][:], pbc[:])
                    else:
                        nc.scalar.copy(es8b_ef[b][h][:], pbc[:])

            # ---------- persistent per-batch tensors ----------
            e1s = [pp.tile([128, MCH, BW], BF16, tag=f"e1s{b}", name=f"e1s{b}")
                   for b in range(B)]
            f1s = [pp.tile([128, NCH, BW], BF16, tag=f"f1s{b}", name=f"f1s{b}")
                   for b in range(B)]
            # t columns and their exps, split in quarter-chunk tiles so the
            # main loop can start after the first quarter of the projections.
            NG = 4
            MH, NH = MCH // NG, NCH // NG
            tfeb = [[pp.tile([128, MH, HEADS], F32, tag=f"tfeb{b}{g}",
                             name=f"tfeb{b}{g}") for g in range(NG)]
                    for b in range(B)]
            tefb = [[pp.tile([128, NH, HEADS], F32, tag=f"tefb{b}{g}",
                             name=f"tefb{b}{g}") for g in range(NG)]
                    for b in range(B)]
            et1_fe = [[[pp.tile([128, MH], F32, tag=f"et1fe{b}{h}{g}",
                                name=f"et1fe{b}{h}{g}") for g in range(NG)]
                       for h in range(HEADS)] for b in range(B)]
            et2_fe = [[[pp.tile([128, MH], F32, tag=f"et2fe{b}{h}{g}",
                                name=f"et2fe{b}{h}{g}") for g in range(NG)]
                       for h in range(HEADS)] for b in range(B)]
            et1_ef = [[[pp.tile([128, NH], F32, tag=f"et1ef{b}{h}{g}",
                                name=f"et1ef{b}{h}{g}") for g in range(NG)]
                       for h in range(HEADS)] for b in range(B)]
            et2_ef = [[[pp.tile([128, NH], F32, tag=f"et2ef{b}{h}{g}",
                                name=f"et2ef{b}{h}{g}") for g in range(NG)]
                       for h in range(HEADS)] for b in range(B)]
            ofe_stage = [[pp.tile([128, HEADS * HDIM], F32, tag=f"ofe{b}{i}",
                                  name=f"ofe{b}{i}") for i in range(NI_FE)]
                         for b in range(B)]
            oef_stage = [[pp.tile([128, HEADS * HDIM], F32, tag=f"oef{b}{i}",
                                  name=f"oef{b}{i}") for i in range(NI_EF)]
                         for b in range(B)]

            e0t_s = [pp.tile([128, N_E], BF16, tag=f"e0t{b}", name=f"e0t{b}")
                     for b in range(B)]
            f0t_s = [pp.tile([128, N_F], BF16, tag=f"f0t{b}", name=f"f0t{b}")
                     for b in range(B)]

            def proj_tile(b, ct, e_side):
                if e_side:
                    src, bundle, dst, tcols, n_half = (
                        e0t_s[b], rhsE, e1s, tfeb, MCH // 4)
                else:
                    src, bundle, dst, tcols, n_half = (
                        f0t_s[b], rhsF, f1s, tefb, NCH // 4)
                p_t = ps.tile([128, BW], F32, tag="proj", name="proj", bufs=2)
                nc.tensor.matmul(p_t[:], src[:, ct * 128:(ct + 1) * 128],
                                 bundle[:], start=True, stop=True)
                # b=0 front is ACT-bound while DVE idles: alternate the
                # PSUM->SBUF copies between the two engines
                tc_view = p_t[:].rearrange("p (h w) -> p h w", w=W)[:, :, HDIM]
                if b == 0 and e_side and ct % 2 == 1:
                    nc.vector.tensor_copy(
                        tcols[b][ct // n_half][:, ct % n_half, :], tc_view)
                    nc.vector.tensor_copy(dst[b][:, ct, :], p_t[:])
                else:
                    nc.scalar.copy(
                        tcols[b][ct // n_half][:, ct % n_half, :], tc_view)
                    nc.scalar.copy(dst[b][:, ct, :], p_t[:])

            def emit_exps(b, g, fe_side):
                tc_t, o1, o2 = ((tfeb, et1_fe, et2_fe) if fe_side
                                else (tefb, et1_ef, et2_ef))
                for h in range(HEADS):
                    nc.scalar.activation(o1[b][h][g][:], tc_t[b][g][:, :, h],
                                         AF.Exp, scale=1.0)
                    nc.scalar.activation(o2[b][h][g][:], tc_t[b][g][:, :, h],
                                         AF.Exp, scale=0.2)

            def emit_projections(b):
                # quarter-split input loads: the first exp group (and so the
                # DVE main loop) unblocks after 1/4 of the transfer
                q = N_E // 4
                for i in range(4):
                    nc.sync.dma_start(e0t_s[b][:, i * q:(i + 1) * q],
                                      e0t[b, :, i * q:(i + 1) * q])
                if b == 0:
                    # adjacency halves queued right after the E projections
                    # input; TT consumes them half by half
                    half = MCH * NFC // 2
                    nc.sync.dma_start(adjfe_s[:, 0:half], adjt_fe[:, 0:half])
                    nc.sync.dma_start(adjfe_s[:, half:], adjt_fe[:, half:])
                q2 = N_F // 2
                for i in range(2):
                    nc.sync.dma_start(f0t_s[b][:, i * q2:(i + 1) * q2],
                                      f0t[b, :, i * q2:(i + 1) * q2])
                if b == 0:
                    half = NCH * NEC // 2
                    nc.sync.dma_start(adjef_s[:, 0:half], adjt_ef[:, 0:half])
                    nc.sync.dma_start(adjef_s[:, half:], adjt_ef[:, half:])
                for g in range(NG):
                    for mt in range(g * MH, (g + 1) * MH):
                        proj_tile(b, mt, True)
                    emit_exps(b, g, True)
                for g in range(NG):
                    for nt in range(g * NH, (g + 1) * NH):
                        proj_tile(b, nt, False)
                    emit_exps(b, g, False)
                # ones columns for the denominator (Pool engine: off the
                # DVE critical path)
                nc.gpsimd.memset(
                    e1s[b][:].rearrange("p c (h w) -> p c h w", w=W)
                    [:, :, :, HDIM], 1.0)
                nc.gpsimd.memset(
                    f1s[b][:].rearrange("p c (h w) -> p c h w", w=W)
                    [:, :, :, HDIM], 1.0)

            def emit_ts_tt(b, h, fe_side):
                if fe_side:
                    nch, ncols, es8b, e1, e2, adj, half = (
                        MCH, NFC, es8b_fe, et1_fe, et2_fe, adjfe_s, MCH // 4)
                else:
                    nch, ncols, es8b, e1, e2, adj, half = (
                        NCH, NEC, es8b_ef, et1_ef, et2_ef, adjef_s, NCH // 4)
                pt = pmain.tile([128, nch * ncols], BF16, tag="P", name="P")
                hf = nch * ncols // 2
                for c in range(nch):
                    g, cg = divmod(c, half)
                    nc.vector.tensor_scalar(
                        pt[:, c * ncols:(c + 1) * ncols], es8b[b][h][:],
                        e1[b][h][g][:, cg:cg + 1], e2[b][h][g][:, cg:cg + 1],
                        ALU.mult, ALU.max)
                nc.vector.tensor_mul(pt[:, 0:hf], pt[:, 0:hf], adj[:, 0:hf])
                nc.vector.tensor_mul(pt[:, hf:], pt[:, hf:], adj[:, hf:])
                return pt

            def emit_matmuls(b, h, pt, fe_side):
                if fe_side:
                    nch, ncols, ni, x1s, potag = MCH, NFC, NI_FE, e1s, "pofe"
                else:
                    nch, ncols, ni, x1s, potag = NCH, NEC, NI_EF, f1s, "poef"
                po = ps.tile([128, ni, W], F32, tag=potag, name=potag, bufs=2)
                for i in range(ni):
                    for c in range(nch):
                        nc.tensor.matmul(
                            po[:, i, :],
                            pt[:, c * ncols + i * 128:c * ncols + (i + 1) * 128],
                            x1s[b][:, c, h * W:(h + 1) * W],
                            start=(c == 0), stop=(c == nch - 1),
                            skip_group_check=True)
                return po

            def emit_finalize(b, h, po, fe_side):
                ni, stage = ((NI_FE, ofe_stage) if fe_side
                             else (NI_EF, oef_stage))
                rec = wp.tile([128, ni], F32, tag="rec", name="rec")
                nc.vector.reciprocal(rec[:], po[:, :, HDIM])
                for i in range(ni):
                    nc.scalar.activation(
                        stage[b][i][:, h * HDIM:(h + 1) * HDIM],
                        po[:, i, 0:HDIM], AF.Copy, scale=rec[:, i:i + 1])

            def emit_stage4(b, fe_side):
                if not fe_side:
                    for i in range(NI_EF):
                        fin = wp.tile([128, HEADS * HDIM], F32, tag="fin",
                                      name="fin")
                        nc.scalar.activation(fin[:], oef_stage[b][i][:],
                                             AF.Relu)
                        nc.sync.dma_start(out_ef[b, i * 128:(i + 1) * 128, :],
                                          fin[:])
                    return
                for i in range(NI_FE):
                    # logits are convex combinations of E1 (|x| <= ~6), so
                    # exp() without the max shift is exact in fp32
                    x = ofe_stage[b][i]
                    ex = wp.tile([128, HEADS * HDIM], F32, tag="ex", name="ex")
                    sm = wp.tile([128, 1], F32, tag="sm", name="sm")
                    nc.scalar.activation(ex[:], x[:], AF.Exp,
                                         scale=1.0, accum_out=sm[:])
                    rc = wp.tile([128, 1], F32, tag="rc", name="rc")
                    nc.vector.reciprocal(rc[:], sm[:])
                    fin = wp.tile([128, HEADS * HDIM], F32, tag="fin",
                                  name="fin")
                    nc.scalar.activation(fin[:], ex[:], AF.Copy, scale=rc[:])
                    nc.sync.dma_start(out_fe[b, i * 128:(i + 1) * 128, :],
                                      fin[:])

            # ---------- main pipeline ----------
            # Phase order: proj(0), fe(0), proj(1), ef(0), fe(1), ef(1).
            # Within a phase, each head's finalize (PSUM reads) is deferred
            # one step so DVE never stalls waiting on PE accumulation.
            pending = []  # deferred finalize closures

            def pop_pending():
                while pending:
                    pending.pop(0)()

            def phase(b, fe_side):
                last_phase = (b == B - 1) and not fe_side
                for h in range(HEADS):
                    pt = emit_ts_tt(b, h, fe_side)
                    pop_pending()
                    if last_phase and h == HEADS - 1:
                        # very last head: interleave finalizes with the
                        # accumulation groups to shorten the tail
                        po = ps.tile([128, NI_EF, W], F32, tag="poef",
                                     name="poef", bufs=2)
                        for i in range(NI_EF):
                            for c in range(NCH):
                                nc.tensor.matmul(
                                    po[:, i, :],
                                    pt[:, c * NEC + i * 128:
                                       c * NEC + (i + 1) * 128],
                                    f1s[b][:, c, h * W:(h + 1) * W],
                                    start=(c == 0), stop=(c == NCH - 1),
                                    skip_group_check=True)
                            rec = wp.tile([128, 1], F32, tag="rec",
                                          name="rec")
                            nc.vector.reciprocal(rec[:], po[:, i, HDIM:W])
                            nc.scalar.activation(
                                oef_stage[b][i][:, h * HDIM:(h + 1) * HDIM],
                                po[:, i, 0:HDIM], AF.Copy, scale=rec[:])
                        emit_stage4(b, False)
                        continue
                    po = emit_matmuls(b, h, pt, fe_side)

                    def fin(b=b, h=h, po=po, fe=fe_side):
                        emit_finalize(b, h, po, fe)
                        if h == HEADS - 1:
                            pending.append(
                                lambda b=b, fe=fe: emit_stage4(b, fe))
                    pending.append(fin)

            for h in range(HEADS):
                emit_wchain(h)
                emit_es8b_h(0, h)
            emit_projections(0)
            for h in range(HEADS):
                emit_es8b_h(1, h)
            phase(0, True)
            emit_projections(1)
            phase(0, False)
            phase(1, True)
            phase(1, False)
            pop_pending()

    nc.compile()
    return nc


_NC_CACHE = None


def _get_program():
    global _NC_CACHE
    if _NC_CACHE is None:
        _NC_CACHE = build_program()
    return _NC_CACHE


def _pack_adj(adj_t_slice, nchunks, ncols):
    # [nchunks*128, ncols] -> [128, nchunks*ncols] with chunk-major free dim
    a = np.ascontiguousarray(adj_t_slice.astype(ml_dtypes.bfloat16))
    a = a.reshape(nchunks, 128, ncols).transpose(1, 0, 2)
    return np.ascontiguousarray(a.reshape(128, nchunks * ncols))


def make_in_maps(F0, E0, adj_fe, adj_ef, Wf, We, a_fe, a_ef):
    F0 = np.asarray(F0, dtype=np.float32)
    E0 = np.asarray(E0, dtype=np.float32)
    Wf = np.asarray(Wf, dtype=np.float32)
    We = np.asarray(We, dtype=np.float32)
    a_fe = np.asarray(a_fe, dtype=np.float32)
    a_ef = np.asarray(a_ef, dtype=np.float32)

    f0t = np.ascontiguousarray(F0.transpose(0, 2, 1)).astype(ml_dtypes.bfloat16)
    e0t = np.ascontiguousarray(E0.transpose(0, 2, 1)).astype(ml_dtypes.bfloat16)
    adjt_fe_full = np.asarray(adj_fe).T.astype(np.float32)  # [N_E, N_F]
    adjt_ef_full = np.asarray(adj_ef).T.astype(np.float32)  # [N_F, N_E]
    wpack = np.zeros((128, 1032), np.float32)
    for h in range(HEADS):
        wpack[0:HDIM, h * 128:(h + 1) * 128] = Wf[h].T
        wpack[HDIM:128, h * 128:(h + 1) * 128] = We[h].T
        wpack[:, 512 + h * HDIM:512 + (h + 1) * HDIM] = Wf[h]
        wpack[:, 768 + h * HDIM:768 + (h + 1) * HDIM] = We[h]
        wpack[0:HDIM, 1024 + h] = a_fe[h]
        wpack[HDIM:128, 1024 + h] = a_fe[h]
        wpack[0:HDIM, 1028 + h] = a_ef[h]
        wpack[HDIM:128, 1028 + h] = a_ef[h]

    in_maps = []
    for k in range(NCORES):
        nsl = slice(k * NFC, (k + 1) * NFC)
        msl = slice(k * NEC, (k + 1) * NEC)
        in_maps.append({
            "adjt_fe": _pack_adj(adjt_fe_full[:, nsl], MCH, NFC),
            "adjt_ef": _pack_adj(adjt_ef_full[:, msl], NCH, NEC),
            "f0t": f0t,
            "e0t": e0t,
            "f0t_own": np.ascontiguousarray(f0t[:, :, nsl]),
            "e0t_own": np.ascontiguousarray(e0t[:, :, msl]),
            "wpack": wpack,
        })
    return in_maps


def kernel(F0, E0, adj_fe, adj_ef, Wf, We, a_fe, a_ef):
    nc = _get_program()
    in_maps = make_in_maps(F0, E0, adj_fe, adj_ef, Wf, We, a_fe, a_ef)
    res = run_bass_kernel_spmd(nc, in_maps, list(range(NCORES)))
    out_fe = np.concatenate([res.results[k]["out_fe"] for k in range(NCORES)],
                            axis=1)
    out_ef = np.concatenate([res.results[k]["out_ef"] for k in range(NCORES)],
                            axis=1)
    return (np.asarray(out_fe, dtype=np.float32),
            np.asarray(out_ef, dtype=np.float32))
